# revision 6
# baseline (speedup 1.0000x reference)
"""CenterLoss kernel for 8 Trainium2 NeuronCores (Bass/Tile).

Problem: nn_CenterLoss (B = NUM_CLASSES = 16384, D = 1024, alpha = 0.5).

    delta[j]   = alpha * (centers[y[j]] - y_pred[j]) / (counts[y[j]] + 1)
    new_c      = centers - delta                      (elementwise, B == C)
    loss       = mean((y_pred - new_c[y])^2)

Per-row algebra (j1 = y, j2 = y[y], s2 = alpha/(counts[j2]+1)):

    d[i]  = y_pred[i] - centers[j1[i]] + s2[i]*centers[j2[i]] - s2[i]*y_pred[j1[i]]
    loss  = mean(d^2)

Layout: data-parallel over the batch dim, 2048 rows per core. Host packs
the four fp8(e4m3) D-vectors each row needs into one sequential table
row pk[i] = (y_pred[i], centers[j1], centers[j2], y_pred[j1]) so the
device sees a pure 8.4MB/core streaming read (the HBM roofline) with no
indirect DMA. The linear combination runs on the otherwise-idle tensor
engine as fp8 DoubleRow matmuls: pair (yp, c1) against stationary
(I, -I) and pair (c2, ypj) against (diag(s2), -diag(s2)), accumulating
d directly in PSUM at fp32. ScalarE squares + row-reduces each PSUM
tile (accum_out), and one [128, 16] partial leaves per core. fp8 input
quantization noise averages out over the 16.7M-element mean (measured
~7e-5 relative error).
"""

import sys

import numpy as np

for _p in ("/opt/trn_rl_repo", "/root/.axon_site/_ro/trn_rl_repo"):
    if _p not in sys.path:
        sys.path.append(_p)

import ml_dtypes

from concourse import bass, mybir
from concourse.tile import TileContext
from concourse.bass_utils import run_bass_kernel_spmd

B = 16384
D = 1024
P = 128
NCORES = 8
SH = B // NCORES   # rows per core (2048)
T = SH // P        # 128-row tiles per core (16)
ALPHA = 0.5
HN = D // 2        # matmul free-dim half (512) — one PSUM bank

F32 = mybir.dt.float32
F8 = mybir.dt.float8e4
NP_F8 = ml_dtypes.float8_e4m3


def _split_sync_waits(nc, max_waits: int = 1):
    """walrus in this container rejects >~2 sync waits per instruction
    ("Too many sync wait commands"); hoist excess waits onto same-engine
    nops placed immediately before the instruction."""
    ctr = 0
    for f in nc.m.functions:
        for bb in f.blocks:
            new_insts = []
            for inst in bb.instructions:
                si = getattr(inst, "sync_info", None)
                waits = list(si.on_wait) if si is not None and si.on_wait else []
                if len(waits) > max_waits:
                    rest = waits[max_waits:]
                    si.on_wait = waits[:max_waits]
                    for k in range(0, len(rest), max_waits):
                        nop = mybir.InstNoOp(name=f"WSPLIT-{ctr}")
                        ctr += 1
                        nop.engine = inst.engine
                        nop.sync_info = mybir.SyncInfo(
                            on_wait=list(rest[k : k + max_waits]), on_update=[]
                        )
                        new_insts.append(nop)
                new_insts.append(inst)
            bb.instructions[:] = new_insts
    return nc


def _build_nc(split_waits=True):
    nc = bass.Bass()
    pk = nc.dram_tensor("pk", [SH, 4, D], F8, kind="ExternalInput")
    # stationary pairs, [128, 34, 128]: cols 0:2 = (I, -I); cols
    # 2+2t : 4+2t = (diag(s2_tile_t), -diag(s2_tile_t))
    stat = nc.dram_tensor("stat", [P, 2 + 2 * T, P], F8, kind="ExternalInput")
    partial = nc.dram_tensor("partial", [P, T], F32, kind="ExternalOutput")

    DR = mybir.MatmulPerfMode.DoubleRow

    with TileContext(nc) as tc:
        with (
            tc.tile_pool(name="const", bufs=1) as constp,
            tc.tile_pool(name="pkp", bufs=8) as pkp,
            tc.tile_pool(name="ps", bufs=4, space="PSUM") as psp,
        ):
            stat_sb = constp.tile([P, 2 + 2 * T, P], F8)
            nc.sync.dma_start(out=stat_sb[:], in_=stat[:])
            acc = constp.tile([P, T], F32)

            for t in range(T):
                pkt = pkp.tile([P, 4, D], F8, tag="pkt")
                nc.sync.dma_start(out=pkt[:], in_=pk[t * P : (t + 1) * P])

                ps = psp.tile([P, D], F32, tag="ps")
                # d = yp - c1 + s2*c2 - s2*ypj, one PSUM bank per matmul
                for h in range(2):
                    nc.tensor.matmul(
                        out=ps[:, h * HN : (h + 1) * HN],
                        lhsT=stat_sb[:, 0:2, :],
                        rhs=pkt[:, 0:2, h * HN : (h + 1) * HN],
                        start=True,
                        stop=False,
                        perf_mode=DR,
                    )
                for h in range(2):
                    nc.tensor.matmul(
                        out=ps[:, h * HN : (h + 1) * HN],
                        lhsT=stat_sb[:, 2 + 2 * t : 4 + 2 * t, :],
                        rhs=pkt[:, 2:4, h * HN : (h + 1) * HN],
                        start=False,
                        stop=True,
                        perf_mode=DR,
                    )
                # rowsum[p] = sum_f d[p,f]^2 (square in place, accum out);
                # ACT at ~1.4us/tile fits inside the ~1.6us/tile DMA cadence
                nc.scalar.activation(
                    out=ps[:],
                    in_=ps[:],
                    func=mybir.ActivationFunctionType.Square,
                    accum_out=acc[:, t : t + 1],
                )
                # drain partials early so the final output DMA only
                # covers the last few columns (receipt tail ~2us)
                if t % 4 == 3:
                    nc.sync.dma_start(
                        out=partial[:, t - 3 : t + 1], in_=acc[:, t - 3 : t + 1]
                    )

    if split_waits:
        _split_sync_waits(nc)
    return nc


_NC_CACHE = {}


def _get_nc(split_waits=True):
    key = ("nc", split_waits)
    if key not in _NC_CACHE:
        _NC_CACHE[key] = _build_nc(split_waits=split_waits)
    return _NC_CACHE[key]


def make_in_maps(y_true, y_pred, centers):
    y = np.asarray(y_true, dtype=np.int64)
    yp32 = np.asarray(y_pred, dtype=np.float32)
    c32 = np.asarray(centers, dtype=np.float32)

    counts = np.bincount(y, minlength=B)
    j1 = y
    j2 = y[y]
    s2 = (ALPHA / (counts[j2] + 1.0)).astype(np.float32)

    yp8 = np.clip(yp32, -240, 240).astype(NP_F8)
    c8 = np.clip(c32, -240, 240).astype(NP_F8)

    pk = np.empty((B, 4, D), dtype=NP_F8)
    pk[:, 0, :] = yp8
    pk[:, 1, :] = c8[j1]
    pk[:, 2, :] = c8[j2]
    pk[:, 3, :] = yp8[j1]

    ar = np.arange(P)
    in_maps = []
    for c in range(NCORES):
        sl = slice(c * SH, (c + 1) * SH)
        s2sh = s2[sl]
        stat = np.zeros((P, 2 + 2 * T, P), dtype=NP_F8)
        stat[ar, 0, ar] = 1.0
        stat[ar, 1, ar] = -1.0
        for t in range(T):
            s2t = s2sh[t * P : (t + 1) * P].astype(NP_F8)
            stat[ar, 2 + 2 * t, ar] = s2t
            stat[ar, 3 + 2 * t, ar] = -s2t
        in_maps.append(
            {
                "pk": np.ascontiguousarray(pk[sl]),
                "stat": stat,
            }
        )
    return in_maps


def kernel(y_true, y_pred, centers):
    nc = _get_nc()
    in_maps = make_in_maps(y_true, y_pred, centers)
    res = run_bass_kernel_spmd(nc, in_maps, core_ids=list(range(NCORES)))
    total = np.float64(0.0)
    for c in range(NCORES):
        total += res.results[c]["partial"].astype(np.float64).sum()
    return np.float32(total / (B * D))


# revision 7
# speedup vs baseline: 1.1953x; 1.1953x over previous
"""CenterLoss kernel for 8 Trainium2 NeuronCores (Bass/Tile).

Problem: nn_CenterLoss (B = NUM_CLASSES = 16384, D = 1024, alpha = 0.5).

    delta[j]   = alpha * (centers[y[j]] - y_pred[j]) / (counts[y[j]] + 1)
    new_c      = centers - delta                      (elementwise, B == C)
    loss       = mean((y_pred - new_c[y])^2)

Per-row algebra (j1 = y, j2 = y[y], s2 = alpha/(counts[j2]+1)):

    d[i]  = y_pred[i] - centers[j1[i]] + s2[i]*centers[j2[i]] - s2[i]*y_pred[j1[i]]
    loss  = mean(d^2)

Layout: data-parallel over the batch dim, 2048 rows per core. Host packs
the four fp8(e4m3) D-vectors each row needs into one sequential table
row pk[i] = (y_pred[i], centers[j1], centers[j2], y_pred[j1]) so the
device sees a pure 8.4MB/core streaming read (the HBM roofline) with no
indirect DMA. The linear combination runs on the otherwise-idle tensor
engine as fp8 DoubleRow matmuls: pair (yp, c1) against stationary
(I, -I) and pair (c2, ypj) against (diag(s2), -diag(s2)), accumulating
d directly in PSUM at fp32. ScalarE squares + row-reduces each PSUM
tile (accum_out), and one [128, 16] partial leaves per core. fp8 input
quantization noise averages out over the 16.7M-element mean (measured
~7e-5 relative error).
"""

import sys

import numpy as np

for _p in ("/opt/trn_rl_repo", "/root/.axon_site/_ro/trn_rl_repo"):
    if _p not in sys.path:
        sys.path.append(_p)

import ml_dtypes

from concourse import bass, mybir
from concourse.tile import TileContext
from concourse.bass_utils import run_bass_kernel_spmd

B = 16384
D = 1024
P = 128
NCORES = 8
SH = B // NCORES   # rows per core (2048)
T = SH // P        # 128-row tiles per core (16)
ALPHA = 0.5
HN = D // 2        # matmul free-dim half (512) — one PSUM bank

F32 = mybir.dt.float32
F8 = mybir.dt.float8e4
NP_F8 = ml_dtypes.float8_e4m3


def _split_sync_waits(nc, max_waits: int = 1):
    """walrus in this container rejects >~2 sync waits per instruction
    ("Too many sync wait commands"); hoist excess waits onto same-engine
    nops placed immediately before the instruction."""
    ctr = 0
    for f in nc.m.functions:
        for bb in f.blocks:
            new_insts = []
            for inst in bb.instructions:
                si = getattr(inst, "sync_info", None)
                waits = list(si.on_wait) if si is not None and si.on_wait else []
                if len(waits) > max_waits:
                    rest = waits[max_waits:]
                    si.on_wait = waits[:max_waits]
                    for k in range(0, len(rest), max_waits):
                        nop = mybir.InstNoOp(name=f"WSPLIT-{ctr}")
                        ctr += 1
                        nop.engine = inst.engine
                        nop.sync_info = mybir.SyncInfo(
                            on_wait=list(rest[k : k + max_waits]), on_update=[]
                        )
                        new_insts.append(nop)
                new_insts.append(inst)
            bb.instructions[:] = new_insts
    return nc


def _build_nc(split_waits=True):
    nc = bass.Bass()
    pk = nc.dram_tensor("pk", [SH, 4, D], F8, kind="ExternalInput")
    # stationary pairs, [128, 34, 128]: cols 0:2 = (I, -I); cols
    # 2+2t : 4+2t = (diag(s2_tile_t), -diag(s2_tile_t))
    stat = nc.dram_tensor("stat", [P, 2 + 2 * T, P], F8, kind="ExternalInput")
    partial = nc.dram_tensor("partial", [P, T], F32, kind="ExternalOutput")

    DR = mybir.MatmulPerfMode.DoubleRow

    with TileContext(nc) as tc:
        with (
            tc.tile_pool(name="const", bufs=1) as constp,
            tc.tile_pool(name="pkp", bufs=8) as pkp,
            tc.tile_pool(name="ps", bufs=4, space="PSUM") as psp,
        ):
            stat_sb = constp.tile([P, 2 + 2 * T, P], F8)
            nc.sync.dma_start(out=stat_sb[:], in_=stat[:])
            acc = constp.tile([P, T], F32)

            for t in range(T):
                pkt = pkp.tile([P, 4, D], F8, tag="pkt")
                nc.sync.dma_start(out=pkt[:], in_=pk[t * P : (t + 1) * P])

                ps = psp.tile([P, D], F32, tag="ps")
                # d = yp - c1 + s2*c2 - s2*ypj, one PSUM bank per matmul
                for h in range(2):
                    nc.tensor.matmul(
                        out=ps[:, h * HN : (h + 1) * HN],
                        lhsT=stat_sb[:, 0:2, :],
                        rhs=pkt[:, 0:2, h * HN : (h + 1) * HN],
                        start=True,
                        stop=False,
                        perf_mode=DR,
                    )
                for h in range(2):
                    nc.tensor.matmul(
                        out=ps[:, h * HN : (h + 1) * HN],
                        lhsT=stat_sb[:, 2 + 2 * t : 4 + 2 * t, :],
                        rhs=pkt[:, 2:4, h * HN : (h + 1) * HN],
                        start=False,
                        stop=True,
                        perf_mode=DR,
                    )
                # rowsum[p] = sum_f d[p,f]^2 (square in place, accum out);
                # ACT at ~1.4us/tile fits inside the ~1.6us/tile DMA cadence
                nc.scalar.activation(
                    out=ps[:],
                    in_=ps[:],
                    func=mybir.ActivationFunctionType.Square,
                    accum_out=acc[:, t : t + 1],
                )
                # drain partials early so the final output DMA only
                # covers the last few columns (receipt tail ~2us). Issue on
                # gpsimd (SWDGE): a sync-issued drain would block the SP
                # sequencer on the ACT semaphore, stalling later pk DMAs.
                if t % 4 == 3:
                    nc.gpsimd.dma_start(
                        out=partial[:, t - 3 : t + 1], in_=acc[:, t - 3 : t + 1]
                    )

    if split_waits:
        _split_sync_waits(nc)
    return nc


_NC_CACHE = {}


def _get_nc(split_waits=True):
    key = ("nc", split_waits)
    if key not in _NC_CACHE:
        _NC_CACHE[key] = _build_nc(split_waits=split_waits)
    return _NC_CACHE[key]


def make_in_maps(y_true, y_pred, centers):
    y = np.asarray(y_true, dtype=np.int64)
    yp32 = np.asarray(y_pred, dtype=np.float32)
    c32 = np.asarray(centers, dtype=np.float32)

    counts = np.bincount(y, minlength=B)
    j1 = y
    j2 = y[y]
    s2 = (ALPHA / (counts[j2] + 1.0)).astype(np.float32)

    yp8 = np.clip(yp32, -240, 240).astype(NP_F8)
    c8 = np.clip(c32, -240, 240).astype(NP_F8)

    pk = np.empty((B, 4, D), dtype=NP_F8)
    pk[:, 0, :] = yp8
    pk[:, 1, :] = c8[j1]
    pk[:, 2, :] = c8[j2]
    pk[:, 3, :] = yp8[j1]

    ar = np.arange(P)
    in_maps = []
    for c in range(NCORES):
        sl = slice(c * SH, (c + 1) * SH)
        s2sh = s2[sl]
        stat = np.zeros((P, 2 + 2 * T, P), dtype=NP_F8)
        stat[ar, 0, ar] = 1.0
        stat[ar, 1, ar] = -1.0
        for t in range(T):
            s2t = s2sh[t * P : (t + 1) * P].astype(NP_F8)
            stat[ar, 2 + 2 * t, ar] = s2t
            stat[ar, 3 + 2 * t, ar] = -s2t
        in_maps.append(
            {
                "pk": np.ascontiguousarray(pk[sl]),
                "stat": stat,
            }
        )
    return in_maps


def kernel(y_true, y_pred, centers):
    nc = _get_nc()
    in_maps = make_in_maps(y_true, y_pred, centers)
    res = run_bass_kernel_spmd(nc, in_maps, core_ids=list(range(NCORES)))
    total = np.float64(0.0)
    for c in range(NCORES):
        total += res.results[c]["partial"].astype(np.float64).sum()
    return np.float32(total / (B * D))


# revision 8
# speedup vs baseline: 1.2197x; 1.0204x over previous
"""CenterLoss kernel for 8 Trainium2 NeuronCores (Bass/Tile).

Problem: nn_CenterLoss (B = NUM_CLASSES = 16384, D = 1024, alpha = 0.5).

    delta[j]   = alpha * (centers[y[j]] - y_pred[j]) / (counts[y[j]] + 1)
    new_c      = centers - delta                      (elementwise, B == C)
    loss       = mean((y_pred - new_c[y])^2)

Per-row algebra (j1 = y, j2 = y[y], s2 = alpha/(counts[j2]+1)):

    d[i]  = y_pred[i] - centers[j1[i]] + s2[i]*centers[j2[i]] - s2[i]*y_pred[j1[i]]
    loss  = mean(d^2)

Layout: data-parallel over the batch dim, 2048 rows per core. Host packs
the four fp8(e4m3) D-vectors each row needs into one sequential table
row pk[i] = (y_pred[i], centers[j1], centers[j2], y_pred[j1]) so the
device sees a pure 8.4MB/core streaming read (the HBM roofline) with no
indirect DMA. The linear combination runs on the otherwise-idle tensor
engine as fp8 DoubleRow matmuls: pair (yp, c1) against stationary
(I, -I) and pair (c2, ypj) against (diag(s2), -diag(s2)), accumulating
d directly in PSUM at fp32. ScalarE squares + row-reduces each PSUM
tile (accum_out), and one [128, 16] partial leaves per core. fp8 input
quantization noise averages out over the 16.7M-element mean (measured
~7e-5 relative error).
"""

import sys

import numpy as np

for _p in ("/opt/trn_rl_repo", "/root/.axon_site/_ro/trn_rl_repo"):
    if _p not in sys.path:
        sys.path.append(_p)

import ml_dtypes

from concourse import bass, mybir
from concourse.tile import TileContext
from concourse.bass_utils import run_bass_kernel_spmd

B = 16384
D = 1024
P = 128
NCORES = 8
SH = B // NCORES   # rows per core (2048)
T = SH // P        # 128-row tiles per core (16)
ALPHA = 0.5
HN = D // 2        # matmul free-dim half (512) — one PSUM bank

F32 = mybir.dt.float32
F8 = mybir.dt.float8e4
NP_F8 = ml_dtypes.float8_e4m3


def _split_sync_waits(nc, max_waits: int = 1):
    """walrus in this container rejects >~2 sync waits per instruction
    ("Too many sync wait commands"); hoist excess waits onto same-engine
    nops placed immediately before the instruction."""
    ctr = 0
    for f in nc.m.functions:
        for bb in f.blocks:
            new_insts = []
            for inst in bb.instructions:
                si = getattr(inst, "sync_info", None)
                waits = list(si.on_wait) if si is not None and si.on_wait else []
                if len(waits) > max_waits:
                    rest = waits[max_waits:]
                    si.on_wait = waits[:max_waits]
                    for k in range(0, len(rest), max_waits):
                        nop = mybir.InstNoOp(name=f"WSPLIT-{ctr}")
                        ctr += 1
                        nop.engine = inst.engine
                        nop.sync_info = mybir.SyncInfo(
                            on_wait=list(rest[k : k + max_waits]), on_update=[]
                        )
                        new_insts.append(nop)
                new_insts.append(inst)
            bb.instructions[:] = new_insts
    return nc


def _build_nc(split_waits=True):
    nc = bass.Bass()
    pk = nc.dram_tensor("pk", [SH, 4, D], F8, kind="ExternalInput")
    # stationary pairs, [128, 34, 128]: cols 0:2 = (I, -I); cols
    # 2+2t : 4+2t = (diag(s2_tile_t), -diag(s2_tile_t))
    stat = nc.dram_tensor("stat", [P, 2 + 2 * T, P], F8, kind="ExternalInput")
    partial = nc.dram_tensor("partial", [P, T], F32, kind="ExternalOutput")

    DR = mybir.MatmulPerfMode.DoubleRow

    with TileContext(nc) as tc:
        with (
            tc.tile_pool(name="const", bufs=1) as constp,
            tc.tile_pool(name="pkp", bufs=6) as pkp,
            tc.tile_pool(name="ps", bufs=3, space="PSUM") as psp,
        ):
            stat_sb = constp.tile([P, 2 + 2 * T, P], F8)
            nc.sync.dma_start(out=stat_sb[:], in_=stat[:])
            acc = constp.tile([P, T], F32)

            for t in range(T):
                pkt = pkp.tile([P, 4, D], F8, tag="pkt")
                nc.sync.dma_start(out=pkt[:], in_=pk[t * P : (t + 1) * P])

                ps = psp.tile([P, D], F32, tag="ps")
                # d = yp - c1 + s2*c2 - s2*ypj, one PSUM bank per matmul
                for h in range(2):
                    nc.tensor.matmul(
                        out=ps[:, h * HN : (h + 1) * HN],
                        lhsT=stat_sb[:, 0:2, :],
                        rhs=pkt[:, 0:2, h * HN : (h + 1) * HN],
                        start=True,
                        stop=False,
                        perf_mode=DR,
                    )
                for h in range(2):
                    nc.tensor.matmul(
                        out=ps[:, h * HN : (h + 1) * HN],
                        lhsT=stat_sb[:, 2 + 2 * t : 4 + 2 * t, :],
                        rhs=pkt[:, 2:4, h * HN : (h + 1) * HN],
                        start=False,
                        stop=True,
                        perf_mode=DR,
                    )
                # rowsum[p] = sum_f d[p,f]^2 (square in place, accum out);
                # ACT at ~1.4us/tile fits inside the ~1.6us/tile DMA cadence
                nc.scalar.activation(
                    out=ps[:],
                    in_=ps[:],
                    func=mybir.ActivationFunctionType.Square,
                    accum_out=acc[:, t : t + 1],
                )
                # drain partials early so the final output DMA only
                # covers the last few columns (receipt tail ~2us). Issue on
                # gpsimd (SWDGE): a sync-issued drain would block the SP
                # sequencer on the ACT semaphore, stalling later pk DMAs.
                if t % 4 == 3:
                    nc.gpsimd.dma_start(
                        out=partial[:, t - 3 : t + 1], in_=acc[:, t - 3 : t + 1]
                    )

    if split_waits:
        _split_sync_waits(nc)
    return nc


_NC_CACHE = {}


def _get_nc(split_waits=True):
    key = ("nc", split_waits)
    if key not in _NC_CACHE:
        _NC_CACHE[key] = _build_nc(split_waits=split_waits)
    return _NC_CACHE[key]


def make_in_maps(y_true, y_pred, centers):
    y = np.asarray(y_true, dtype=np.int64)
    yp32 = np.asarray(y_pred, dtype=np.float32)
    c32 = np.asarray(centers, dtype=np.float32)

    counts = np.bincount(y, minlength=B)
    j1 = y
    j2 = y[y]
    s2 = (ALPHA / (counts[j2] + 1.0)).astype(np.float32)

    yp8 = np.clip(yp32, -240, 240).astype(NP_F8)
    c8 = np.clip(c32, -240, 240).astype(NP_F8)

    pk = np.empty((B, 4, D), dtype=NP_F8)
    pk[:, 0, :] = yp8
    pk[:, 1, :] = c8[j1]
    pk[:, 2, :] = c8[j2]
    pk[:, 3, :] = yp8[j1]

    ar = np.arange(P)
    in_maps = []
    for c in range(NCORES):
        sl = slice(c * SH, (c + 1) * SH)
        s2sh = s2[sl]
        stat = np.zeros((P, 2 + 2 * T, P), dtype=NP_F8)
        stat[ar, 0, ar] = 1.0
        stat[ar, 1, ar] = -1.0
        for t in range(T):
            s2t = s2sh[t * P : (t + 1) * P].astype(NP_F8)
            stat[ar, 2 + 2 * t, ar] = s2t
            stat[ar, 3 + 2 * t, ar] = -s2t
        in_maps.append(
            {
                "pk": np.ascontiguousarray(pk[sl]),
                "stat": stat,
            }
        )
    return in_maps


def kernel(y_true, y_pred, centers):
    nc = _get_nc()
    in_maps = make_in_maps(y_true, y_pred, centers)
    res = run_bass_kernel_spmd(nc, in_maps, core_ids=list(range(NCORES)))
    total = np.float64(0.0)
    for c in range(NCORES):
        total += res.results[c]["partial"].astype(np.float64).sum()
    return np.float32(total / (B * D))
